# revision 1
# baseline (speedup 1.0000x reference)
"""Causal self-attention with Music-Transformer relative position, on 8 TRN2 cores.

v2: transposed-S formulation. S^T[j, i] tiles are computed directly
(lhsT = K-chunk), so attention@V consumes P^T with no DMA transposes of P.
The skewed exp(QEr) factor arrives via a *fused* skew-read + xbar-transpose
from the padded DRAM buffer; the xbar's deterministic row interleave
(out[p, ch, f] = in[f*c + ch, p]) is undone inside the DVE multiply with a
3-dim access pattern. Row sums come free from a ones-column appended to the
V weights (M=65 matmuls); normalization happens in the yT write-back.

Sharding: core c -> batch b = c // 4, heads [4*(c%4), 4*(c%4)+4) (2 pairs).
Host sums the 8 row-parallel projection partials.
"""

import numpy as np
from contextlib import ExitStack

import concourse.bass as bass
import concourse.tile as tile
from concourse import mybir, bacc
from concourse.bass_utils import run_bass_kernel_spmd

F32 = mybir.dt.float32
F16 = mybir.dt.float16

B, L, D = 2, 2048, 1024
NH, HS = 16, 64
BLOCK_SIZE = 2048
SCALE = 1.0 / 8.0
LP = L + 512
N_CORES = 8
HPC = 4

EXP = mybir.ActivationFunctionType.Exp
COPY = mybir.ActivationFunctionType.Copy
MULT = mybir.AluOpType.mult
BYPASS = mybir.AluOpType.bypass


def _build_program():
    nc = bacc.Bacc("TRN2", target_bir_lowering=False, debug=False)

    xT_d = nc.dram_tensor("xT", [D, L], F16, kind="ExternalInput")
    wq_d = nc.dram_tensor("wq", [2, D, 128], F16, kind="ExternalInput")
    wk_d = nc.dram_tensor("wk", [2, D, 128], F16, kind="ExternalInput")
    wv_d = nc.dram_tensor("wv", [2, D, 128], F16, kind="ExternalInput")
    bq_d = nc.dram_tensor("bq", [2, 128, 1], F32, kind="ExternalInput")
    bk_d = nc.dram_tensor("bk", [2, 128, 1], F32, kind="ExternalInput")
    erT_d = nc.dram_tensor("erT", [HS, L], F16, kind="ExternalInput")
    wproj_d = nc.dram_tensor("wproj", [2, 128, D], F16, kind="ExternalInput")
    out_d = nc.dram_tensor("out", [L, D], F16, kind="ExternalOutput")

    # per-(pair, head) padded exp(QEr/8) scratch
    eqr_d = [nc.dram_tensor(f"eqr{u}", [L, LP], F16, kind="Internal") for u in range(4)]

    with tile.TileContext(nc) as tc, ExitStack() as ctx:
        persist = ctx.enter_context(tc.tile_pool(name="persist", bufs=1))
        qT = [persist.tile([128, L], F16, tag=f"qT{p}", name=f"qT{p}") for p in range(2)]
        kT = [persist.tile([128, L], F16, tag=f"kT{p}", name=f"kT{p}") for p in range(2)]
        vT = [persist.tile([128, L], F16, tag=f"vT{p}", name=f"vT{p}") for p in range(2)]
        # vj[p][jc]: [128 j, 132] = [s0 64d | ones | pad | s1 64d | ones | pad]
        vj = [[persist.tile([128, 132], F16, tag=f"vj{p}_{jc}", name=f"vj{p}_{jc}")
               for jc in range(16)] for p in range(2)]
        yT = [persist.tile([128, L], F16, tag=f"yT{p}", name=f"yT{p}") for p in range(2)]
        wproj_sb = persist.tile([128, 2, D], F16, tag="wproj")
        bq_sb = persist.tile([128, 2], F32, tag="bq")
        bk_sb = persist.tile([128, 2], F32, tag="bk")
        erT_sb = persist.tile([128, L], F16, tag="erT")
        ident = persist.tile([128, 128], F16, tag="ident")
        from concourse.masks import make_identity
        make_identity(nc, ident[:, :])
        nc.sync.dma_start(wproj_sb[:, 0, :], wproj_d.ap()[0])
        nc.sync.dma_start(wproj_sb[:, 1, :], wproj_d.ap()[1])
        nc.sync.dma_start(bq_sb[:, 0:1], bq_d.ap()[0])
        nc.sync.dma_start(bq_sb[:, 1:2], bq_d.ap()[1])
        nc.sync.dma_start(bk_sb[:, 0:1], bk_d.ap()[0])
        nc.sync.dma_start(bk_sb[:, 1:2], bk_d.ap()[1])
        nc.sync.dma_start(erT_sb[0:HS, :], erT_d.ap())
        nc.sync.dma_start(erT_sb[HS:128, :], erT_d.ap())

        # ---------- phase 1: qkv projection ----------
        with tc.tile_pool(name="ph1", bufs=1) as ph1, \
             tc.tile_pool(name="ph1ps", bufs=3, space="PSUM") as ph1ps:
            xT_sb = ph1.tile([128, 8, L], F16)
            w_sb = {}
            for name, d_ in (("q", wq_d), ("k", wk_d), ("v", wv_d)):
                w_sb[name] = ph1.tile([128, 2, 8, 128], F16, tag=f"w{name}", name=f"w{name}sb")
                nc.sync.dma_start(
                    w_sb[name][:, :, :, :],
                    d_.ap().rearrange("pp (kc part) col -> part pp kc col", part=128),
                )
            for half in range(2):
                nc.sync.dma_start(
                    xT_sb[:, 4 * half : 4 * half + 4, :],
                    xT_d.ap()[512 * half : 512 * half + 512, :].rearrange(
                        "(kc part) col -> part kc col", part=128
                    ),
                )
            def emit_qkv(p):
                for ib in range(4):
                    isl = bass.ts(ib, 512)
                    for name in ("q", "k", "v"):
                        ps_t = ph1ps.tile([128, 512], F32, tag="qkv")
                        for kc in range(8):
                            nc.tensor.matmul(
                                ps_t[:, :],
                                lhsT=w_sb[name][:, p, kc, :],
                                rhs=xT_sb[:, kc, isl],
                                start=(kc == 0),
                                stop=(kc == 7),
                            )
                        if name == "q":
                            nc.vector.tensor_scalar_add(qT[p][:, isl], ps_t[:, :], bq_sb[:, p : p + 1])
                        elif name == "k":
                            nc.vector.tensor_scalar_add(kT[p][:, isl], ps_t[:, :], bk_sb[:, p : p + 1])
                        else:
                            nc.vector.tensor_copy(vT[p][:, isl], ps_t[:, :])

            emit_qkv(0)
            emit_qkv(1)

        # ---------- vj build: V[j, d] via wide exact xbar transpose + ones col ----------
        with tc.tile_pool(name="vtrp", bufs=1) as vtrp:
            for p in range(2):
                vtr = vtrp.tile([128, 16, 128], F16, tag=f"vtr{p}", name=f"vtr{p}")
                nc.sync.dma_start_transpose(vtr[:, :, :], vT[p][:, :])
                for jc in range(16):
                    nc.vector.tensor_copy(vj[p][jc][:, 0:64], vtr[:, jc, 0:64])
                    nc.vector.tensor_copy(vj[p][jc][:, 66:130], vtr[:, jc, 64:128])
                    nc.vector.memset(vj[p][jc][:, 64:65], 1.0)
                    nc.vector.memset(vj[p][jc][:, 130:131], 1.0)

        # ---------- phases 2+3+attv per (p, s) unit ----------
        with tc.tile_pool(name="p2ps", bufs=3, space="PSUM") as p2ps, \
             tc.tile_pool(name="stps", bufs=3, space="PSUM") as stps, \
             tc.tile_pool(name="avps", bufs=2, space="PSUM") as avps, \
             tc.tile_pool(name="eqp", bufs=6) as eqp, \
             tc.tile_pool(name="esp", bufs=6) as esp, \
             tc.tile_pool(name="ptp", bufs=1) as ptp, \
             tc.tile_pool(name="rsp", bufs=2) as rsp, \
             tc.tile_pool(name="outp", bufs=3) as outp:

            def emit_qer(u):
                p, s = u // 2, u % 2
                sl = slice(64 * s, 64 * s + 64)
                for ib in range(15, -1, -1):
                    i0 = ib * 128
                    mlo = (L - 128 - i0) // 512
                    eq = eqp.tile([128, LP], F16, tag="eq", name=f"eq{u}_{ib}")
                    nc.vector.memset(eq[:, L:LP], 0.0)
                    for mb in range(mlo, 4):
                        ps_t = p2ps.tile([128, 512], F32, tag="qer", name=f"qerps{u}_{ib}_{mb}")
                        nc.tensor.matmul(
                            ps_t[:, :],
                            lhsT=qT[p][sl, i0 : i0 + 128],
                            rhs=erT_sb[sl, bass.ts(mb, 512)],
                            start=True,
                            stop=True,
                            tile_position=(64 * s, 0),
                        )
                        nc.scalar.activation(eq[:, bass.ts(mb, 512)], ps_t[:, :], EXP, scale=SCALE)
                    nc.scalar.dma_start(
                        eqr_d[u].ap()[i0 : i0 + 128, mlo * 512 : LP],
                        eq[:, mlo * 512 : LP],
                    )

            def emit_st(u):
                """S^T tiles: pt[(s, jc)][j 128, i N] = exp(QK^T/8) * skew-es."""
                p, s = u // 2, u % 2
                sl = slice(64 * s, 64 * s + 64)
                pts = {}
                ess = {}
                for jc in range(15, -1, -1):
                    i0 = 512 * (jc // 4)
                    N = L - i0
                    # fused skew-read + exact transpose from DRAM
                    es = esp.tile([128, 2048], F16, tag="es", name=f"es{u}_{jc}")
                    ess[jc] = es
                    src = bass.AP(
                        tensor=eqr_d[u],
                        offset=(L - 1) + i0 * (LP - 1) + 128 * jc,
                        ap=[[LP - 1, N], [1, 128]],
                    )
                    nc.sync.dma_start_transpose(
                        es[:, 0:N].rearrange("p (ch f) -> p ch f", f=128), src
                    )
                for jc in range(15, -1, -1):
                    i0 = 512 * (jc // 4)
                    N = L - i0
                    pt = ptp.tile([128, N], F16, tag=f"pt{s}_{jc}", name=f"pt{s}_{jc}")
                    pts[jc] = pt
                    es = ess[jc]
                    for nb in range(N // 512):
                        ps_t = stps.tile([128, 512], F32, tag="st", name=f"stps{u}_{jc}_{nb}")
                        nc.tensor.matmul(
                            ps_t[:, :],
                            lhsT=kT[p][sl, bass.ts(jc, 128)],
                            rhs=qT[p][sl, i0 + 512 * nb : i0 + 512 * nb + 512],
                            start=True,
                            stop=True,
                            tile_position=(64 * s, 0),
                        )
                        nc.scalar.activation(
                            pt[:, bass.ts(nb, 512)], ps_t[:, :], EXP, scale=SCALE
                        )
                    # in-place multiply by es with the xbar interleave undone:
                    # es memory pos ch*128 + f holds skew row f*c + ch = natural i
                    # iterate (d1 in [0,c), d2 in [0,128)):
                    #   pt col = d1 + c*d2 ; es pos = 128*d1 + d2
                    nc.vector.tensor_tensor(pt[:, 0:N], pt[:, 0:N], es[:, 0:N], MULT)
                return pts

            def emit_attv(u, pts, proj_cb=None):
                p, s = u // 2, u % 2
                sl = slice(64 * s, 64 * s + 64)
                for IB in range(4):
                    ps_y = avps.tile([65, 512], F32, tag="yt", name=f"yt{u}_{IB}")
                    njc = 4 * (IB + 1)
                    for jc in range(njc):
                        i0 = 512 * (jc // 4)
                        off = 512 * IB - i0
                        nc.tensor.matmul(
                            ps_y[:, :],
                            lhsT=vj[p][jc][:, 66 * s : 66 * s + 65],
                            rhs=pts[jc][:, off : off + 512],
                            start=(jc == 0),
                            stop=(jc == njc - 1),
                        )
                    rs = rsp.tile([1, 512], F32, tag="rs", name=f"rs{u}_{IB}")
                    nc.vector.reciprocal(rs[:, :], ps_y[64:65, :])
                    rb = rsp.tile([64, 512], F32, tag="rb", name=f"rb{u}_{IB}")
                    nc.gpsimd.partition_broadcast(rb[:, :], rs[0:1, :])
                    nc.vector.scalar_tensor_tensor(
                        out=yT[p][sl, bass.ts(IB, 512)],
                        in0=ps_y[0:64, :],
                        scalar=1.0,
                        in1=rb[:, :],
                        op0=BYPASS,
                        op1=MULT,
                    )
                    if proj_cb is not None:
                        proj_cb(IB)

            def emit_proj(IB):
                for i128 in range(4 * IB, 4 * IB + 4):
                    o_t = outp.tile([128, 1024], F16, tag="out", name=f"out{i128}")
                    for eb in range(2):
                        ps_o = stps.tile([128, 512], F32, tag="st", name=f"proj{i128}_{eb}")
                        for p in range(2):
                            nc.tensor.matmul(
                                ps_o[:, :],
                                lhsT=yT[p][:, bass.ts(i128, 128)],
                                rhs=wproj_sb[:, p, bass.ts(eb, 512)],
                                start=(p == 0),
                                stop=(p == 1),
                            )
                        nc.vector.tensor_copy(o_t[:, bass.ts(eb, 512)], ps_o[:, :])
                    nc.sync.dma_start(out_d.ap()[bass.ts(i128, 128), :], o_t[:, :])

            emit_qer(0)
            emit_qer(1)
            prev = None
            for u in range(4):
                if u + 2 < 4:
                    emit_qer(u + 2)
                pts = emit_st(u)
                if prev is not None:
                    emit_attv(*prev)
                prev = (u, pts)
            emit_attv(*prev, proj_cb=emit_proj)

    nc.compile()
    return nc


_NC_CACHE = None
RUN_KWARGS = {}
LAST_RESULTS = None
LAST_IN_MAPS = None


def _get_program():
    global _NC_CACHE
    if _NC_CACHE is None:
        _NC_CACHE = _build_program()
    return _NC_CACHE


def kernel(x, Wqkv, bqkv, Wproj, bproj, Er):
    x = np.asarray(x, dtype=np.float32)
    Wqkv = np.asarray(Wqkv, dtype=np.float32)
    bqkv = np.asarray(bqkv, dtype=np.float32)
    Wproj = np.asarray(Wproj, dtype=np.float32)
    bproj = np.asarray(bproj, dtype=np.float32)
    Er = np.asarray(Er, dtype=np.float32)

    nc = _get_program()

    start = BLOCK_SIZE - L
    erT = np.ascontiguousarray(Er[start:, :].T).astype(np.float16)

    in_maps = []
    for c in range(N_CORES):
        b = c // 4
        h0 = HPC * (c % 4)
        xT = np.ascontiguousarray(x[b].T).astype(np.float16)
        wq = np.empty((2, D, 128), np.float16)
        wk = np.empty((2, D, 128), np.float16)
        wv = np.empty((2, D, 128), np.float16)
        bq = np.empty((2, 128, 1), np.float32)
        bk = np.empty((2, 128, 1), np.float32)
        wproj = np.empty((2, 128, D), np.float16)
        for p in range(2):
            c0 = (h0 + 2 * p) * HS
            wq[p] = Wqkv[:, c0 : c0 + 128]
            wk[p] = Wqkv[:, D + c0 : D + c0 + 128]
            wv[p] = Wqkv[:, 2 * D + c0 : 2 * D + c0 + 128]
            bq[p, :, 0] = bqkv[c0 : c0 + 128]
            bk[p, :, 0] = bqkv[D + c0 : D + c0 + 128]
            wproj[p] = Wproj[c0 : c0 + 128, :].astype(np.float16)
        in_maps.append(
            {"xT": xT, "wq": wq, "wk": wk, "wv": wv, "bq": bq, "bk": bk,
             "erT": erT, "wproj": wproj}
        )

    global LAST_RESULTS, LAST_IN_MAPS
    LAST_IN_MAPS = in_maps
    res = run_bass_kernel_spmd(nc, in_maps, core_ids=list(range(N_CORES)), **RUN_KWARGS)
    LAST_RESULTS = res

    bv = bqkv[2 * D :]
    bias_vec = bv @ Wproj + bproj
    out = np.zeros((B, L, D), np.float32)
    for c in range(N_CORES):
        out[c // 4] += res.results[c]["out"].astype(np.float32)
    out += bias_vec[None, None, :]
    return out



# revision 3
# speedup vs baseline: 3.8494x; 3.8494x over previous
"""Causal self-attention with Music-Transformer relative position, on 8 TRN2 cores.

v6: single-exp formulation. Raw QEr logits (exact width, 128-granular) are
stored to DRAM (fp16, with a -30000 128-col pad block implementing the causal
mask), skew-read back via the xbar-transpose DMA (which lands in natural
[j, i] order), ADDED to the QK^T PSUM tile via a PE identity-matmul
accumulate (keeps the produce chain single-engine), and exp'd ONCE on ACT —
halving Activation work vs exp(QEr)*exp(QK). Fully-masked lead columns of
each S^T tile (i < 128*jc) are skipped in the matmul/read/exp and zero-filled.

Schedule: Q projections first, then the four QEr units up-front interleaved
with the K/V projections (their evac/store/skew-read pipelines drain while PE
does useful work), then the software-pipelined S^T -> attV -> proj loop.
Skew-reads are issued per-ib as stores land so each es tile becomes ready as
early as possible; row sums ride along as a ones-column in the V weights.

Engine assignment: PE all matmuls + es-adds; QEr PSUM evacuation 1/3 ACT,
2/3 DVE (GPSIMD cannot access PSUM on this target); ACT exp + proj/out
evacuation; DVE evacuation majority + qkv bias-adds + softmax-normalize;
Pool (gpsimd) SBUF-only side work (vj build, lead memsets, 1/Z partition
broadcast). PSUM: qkv 2 + qer 6 banks early, st 4 + attv 3 late.

Sharding: core c -> batch b = c // 4, heads [4*(c%4), 4*(c%4)+4) (2 pairs).
Host sums the 8 row-parallel projection partials.
"""

import numpy as np
from contextlib import ExitStack

import concourse.bass as bass
import concourse.tile as tile
from concourse import mybir, bacc
from concourse.bass_utils import run_bass_kernel_spmd

F32 = mybir.dt.float32
F16 = mybir.dt.float16

B, L, D = 2, 2048, 1024
NH, HS = 16, 64
BLOCK_SIZE = 2048
SCALE = 1.0 / 8.0
# lead-trimmed skew reads satisfy i >= 128*jc, so the max masked-read column
# is L+126 — a 128-col pad suffices
LP = L + 128
N_CORES = 8
HPC = 4
NEG = -30000.0

EXP = mybir.ActivationFunctionType.Exp
COPY = mybir.ActivationFunctionType.Copy
MULT = mybir.AluOpType.mult
ADD = mybir.AluOpType.add
BYPASS = mybir.AluOpType.bypass

# per-u engine choices: QEr PSUM evacuation, es-add
EVAC_ENGINE = ["act", "act", "act", "dve"]
ADD_ENGINE = ["pe", "pe", "pe", "pe"]


def _build_program():
    nc = bacc.Bacc("TRN2", target_bir_lowering=False, debug=False)

    xT_d = nc.dram_tensor("xT", [D, L], F16, kind="ExternalInput")
    wq_d = nc.dram_tensor("wq", [2, D, 128], F16, kind="ExternalInput")
    wk_d = nc.dram_tensor("wk", [2, D, 128], F16, kind="ExternalInput")
    wv_d = nc.dram_tensor("wv", [2, D, 128], F16, kind="ExternalInput")
    bq_d = nc.dram_tensor("bq", [2, 128, 1], F32, kind="ExternalInput")
    bk_d = nc.dram_tensor("bk", [2, 128, 1], F32, kind="ExternalInput")
    erT_d = nc.dram_tensor("erT", [HS, L], F16, kind="ExternalInput")
    wproj_d = nc.dram_tensor("wproj", [2, 128, D], F16, kind="ExternalInput")
    out_d = nc.dram_tensor("out", [L, D], F16, kind="ExternalOutput")

    # per-(pair, head) padded raw QEr scratch
    eqr_d = [nc.dram_tensor(f"eqr{u}", [L, LP], F16, kind="Internal") for u in range(4)]

    with tile.TileContext(nc) as tc, ExitStack() as ctx:
        persist = ctx.enter_context(tc.tile_pool(name="persist", bufs=1))
        qT = [persist.tile([128, L], F16, tag=f"qT{p}", name=f"qT{p}") for p in range(2)]
        kT = [persist.tile([128, L], F16, tag=f"kT{p}", name=f"kT{p}") for p in range(2)]
        vT = [persist.tile([128, L], F16, tag=f"vT{p}", name=f"vT{p}") for p in range(2)]
        # vj[p][jc]: [128 j, 132] = [s0 64d | ones | pad | s1 64d | ones | pad]
        vj = [[persist.tile([128, 132], F16, tag=f"vj{p}_{jc}", name=f"vj{p}_{jc}")
               for jc in range(16)] for p in range(2)]
        yT = [persist.tile([128, L], F16, tag=f"yT{p}", name=f"yT{p}") for p in range(2)]
        wproj_sb = persist.tile([128, 2, D], F16, tag="wproj")
        bq_sb = persist.tile([128, 2], F32, tag="bq")
        bk_sb = persist.tile([128, 2], F32, tag="bk")
        erT_sb = persist.tile([128, L], F16, tag="erT")
        ident = persist.tile([128, 128], F16, tag="ident")
        from concourse.masks import make_identity
        make_identity(nc, ident[:, :])
        # raw-QEr staging buffers: pad [L:LP) pre-set to NEG once, rotated manually
        NEQ = 10
        eqbuf = [persist.tile([128, LP], F16, tag=f"eqb{r}", name=f"eqb{r}")
                 for r in range(NEQ)]
        for r in range(NEQ):
            nc.gpsimd.memset(eqbuf[r][:, L:LP], NEG)

        eqrot = [0]
        esp = ctx.enter_context(tc.tile_pool(name="esp", bufs=6))
        es_tiles = {}

        def emit_qer(u, evac):
            p, s = u // 2, u % 2
            sl = slice(64 * s, 64 * s + 64)
            for ib in range(15, -1, -1):
                evac = "act" if (u * 16 + ib) % 3 == 0 else "dve"
                i0 = ib * 128
                lo = ((L - 128 - i0) // 128) * 128   # exact 128-granular low edge
                eq = eqbuf[eqrot[0] % NEQ]
                eqrot[0] += 1
                c = lo
                while c < L:
                    w = min(512, L - c)
                    ps_t = qerps.tile([128, 512], F32, tag="qer", name=f"qerps{u}_{ib}_{c}")
                    nc.tensor.matmul(
                        ps_t[:, 0:w],
                        lhsT=qT[p][sl, i0 : i0 + 128],
                        rhs=erT_sb[sl, c : c + w],
                        start=True,
                        stop=True,
                        tile_position=(64 * s, 0),
                    )
                    if evac == "act":
                        nc.scalar.activation(eq[:, c : c + w], ps_t[:, 0:w], COPY)
                    else:
                        nc.vector.tensor_copy(eq[:, c : c + w], ps_t[:, 0:w])
                    c += w
                nc.scalar.dma_start(
                    eqr_d[u].ap()[i0 : i0 + 128, lo:LP],
                    eq[:, lo:LP],
                )
                # skew-read for jc == ib becomes legal once stores ib..15 done
                jc = ib
                i_start = 128 * jc
                Nr = L - i_start
                es = esp.tile([128, 2048], F16, tag="es", name=f"es{u}_{jc}")
                es_tiles[(u, jc)] = es
                src = bass.AP(
                    tensor=eqr_d[u],
                    offset=(L - 1) + i_start * (LP - 1) + 128 * jc,
                    ap=[[LP - 1, Nr], [1, 128]],
                )
                nc.sync.dma_start_transpose(
                    es[:, 0:Nr].rearrange("p (ch f) -> p ch f", f=128), src
                )

        with tc.tile_pool(name="ph1", bufs=1) as ph1, \
             tc.tile_pool(name="ph1ps", bufs=2, space="PSUM") as ph1ps:
            # x loaded as 4 independent 512-col slice tiles so q matmuls can
            # start after the first slice lands; wq issued first
            w_sb = {}
            xts = [ph1.tile([128, 8, 512], F16, tag=f"xts{ib}", name=f"xts{ib}")
                   for ib in range(4)]
            for name, d_ in (("q", wq_d), ("k", wk_d), ("v", wv_d)):
                w_sb[name] = ph1.tile([128, 2, 8, 128], F16, tag=f"w{name}", name=f"w{name}sb")
            nc.sync.dma_start(
                w_sb["q"][:, :, :, :],
                wq_d.ap().rearrange("pp (kc part) col -> part pp kc col", part=128),
            )
            for ib in range(4):
                nc.sync.dma_start(
                    xts[ib][:, :, :],
                    xT_d.ap()[:, bass.ts(ib, 512)].rearrange(
                        "(kc part) col -> part kc col", part=128
                    ),
                )
            nc.sync.dma_start(bq_sb[:, 0:1], bq_d.ap()[0])
            nc.sync.dma_start(bq_sb[:, 1:2], bq_d.ap()[1])
            nc.sync.dma_start(bk_sb[:, 0:1], bk_d.ap()[0])
            nc.sync.dma_start(bk_sb[:, 1:2], bk_d.ap()[1])
            nc.sync.dma_start(erT_sb[0:HS, :], erT_d.ap())
            nc.sync.dma_start(erT_sb[HS:128, :], erT_d.ap())
            for name, d_ in (("k", wk_d), ("v", wv_d)):
                nc.sync.dma_start(
                    w_sb[name][:, :, :, :],
                    d_.ap().rearrange("pp (kc part) col -> part pp kc col", part=128),
                )
            nc.sync.dma_start(wproj_sb[:, 0, :], wproj_d.ap()[0])
            nc.sync.dma_start(wproj_sb[:, 1, :], wproj_d.ap()[1])

            def emit_proj_part(p, name, ib):
                ps_t = ph1ps.tile([128, 512], F32, tag="qkv")
                for kc in range(8):
                    nc.tensor.matmul(
                        ps_t[:, :],
                        lhsT=w_sb[name][:, p, kc, :],
                        rhs=xts[ib][:, kc, :],
                        start=(kc == 0),
                        stop=(kc == 7),
                    )
                isl = bass.ts(ib, 512)
                if name == "q":
                    nc.vector.tensor_scalar_add(qT[p][:, isl], ps_t[:, :], bq_sb[:, p : p + 1])
                elif name == "k":
                    nc.vector.tensor_scalar_add(kT[p][:, isl], ps_t[:, :], bk_sb[:, p : p + 1])
                else:
                    nc.vector.tensor_copy(vT[p][:, isl], ps_t[:, :])

            def emit_vj(p):
                vtr = ph1.tile([128, 16, 128], F16, tag=f"vtr{p}", name=f"vtr{p}")
                nc.sync.dma_start_transpose(vtr[:, :, :], vT[p][:, :])
                for jc in range(16):
                    nc.gpsimd.tensor_copy(vj[p][jc][:, 0:64], vtr[:, jc, 0:64])
                    nc.gpsimd.tensor_copy(vj[p][jc][:, 66:130], vtr[:, jc, 64:128])
                    nc.gpsimd.memset(vj[p][jc][:, 64:65], 1.0)
                    nc.gpsimd.memset(vj[p][jc][:, 130:131], 1.0)

            # ---- q for both pairs, then all QEr units, then k/v ----
            for p in range(2):
                for ib in range(4):
                    emit_proj_part(p, "q", ib)
            with tc.tile_pool(name="qerps", bufs=6, space="PSUM") as qerps:
                def emit_kv(p):
                    for ib in range(4):
                        emit_proj_part(p, "k", ib)
                        emit_proj_part(p, "v", ib)

                emit_qer(0, EVAC_ENGINE[0])
                emit_qer(1, EVAC_ENGINE[1])
                emit_kv(0)
                emit_vj(0)
                emit_qer(2, EVAC_ENGINE[2])
                emit_qer(3, EVAC_ENGINE[3])
                emit_kv(1)
                emit_vj(1)

        # ---------- phases st+attv+proj per (p, s) unit ----------
        with tc.tile_pool(name="stps", bufs=4, space="PSUM") as stps, \
             tc.tile_pool(name="avps", bufs=3, space="PSUM") as avps, \
             tc.tile_pool(name="ptp", bufs=1) as ptp, \
             tc.tile_pool(name="rsp", bufs=1) as rsp, \
             tc.tile_pool(name="outp", bufs=3) as outp:

            def emit_st(u, addeng):
                """S^T tiles: pt[jc][j 128, i N] = exp((QK^T + skew-QEr)/8)."""
                p, s = u // 2, u % 2
                sl = slice(64 * s, 64 * s + 64)
                pts = {}
                for jc in range(15, -1, -1):
                    i0 = 512 * (jc // 4)
                    i_start = 128 * jc
                    N = L - i0
                    r = jc % 4                  # lead cols [i0, i_start) are masked
                    pt = ptp.tile([128, N], F16, tag=f"pt{s}_{jc}", name=f"pt{s}_{jc}")
                    pts[jc] = pt
                    es = es_tiles[(u, jc)]
                    if r:
                        nc.gpsimd.memset(pt[:, 0 : 128 * r], 0.0)
                    for nb in range(N // 512):
                        lead = 128 * r if nb == 0 else 0
                        w = 512 - lead
                        ps_t = stps.tile([128, 512], F32, tag="st", name=f"stps{u}_{jc}_{nb}")
                        es_off = i0 + 512 * nb + lead - i_start
                        if addeng == "pe":
                            nc.tensor.matmul(
                                ps_t[:, lead:512],
                                lhsT=kT[p][sl, bass.ts(jc, 128)],
                                rhs=qT[p][sl, i0 + 512 * nb + lead : i0 + 512 * (nb + 1)],
                                start=True,
                                stop=False,
                                tile_position=(64 * s, 0),
                            )
                            nc.tensor.matmul(
                                ps_t[:, lead:512],
                                lhsT=ident[:, :],
                                rhs=es[:, es_off : es_off + w],
                                start=False,
                                stop=True,
                            )
                        else:
                            nc.tensor.matmul(
                                ps_t[:, lead:512],
                                lhsT=kT[p][sl, bass.ts(jc, 128)],
                                rhs=qT[p][sl, i0 + 512 * nb + lead : i0 + 512 * (nb + 1)],
                                start=True,
                                stop=True,
                                tile_position=(64 * s, 0),
                            )
                            nc.vector.tensor_tensor(
                                ps_t[:, lead:512], ps_t[:, lead:512],
                                es[:, es_off : es_off + w], ADD,
                            )
                        nc.scalar.activation(
                            pt[:, 512 * nb + lead : 512 * (nb + 1)],
                            ps_t[:, lead:512], EXP, scale=SCALE,
                        )
                return pts

            def emit_attv(u, pts, proj_cb=None):
                p, s = u // 2, u % 2
                sl = slice(64 * s, 64 * s + 64)
                for IB in range(4):
                    ps_y = avps.tile([65, 512], F32, tag="yt", name=f"yt{u}_{IB}")
                    njc = 4 * (IB + 1)
                    for jc in range(njc):
                        i0 = 512 * (jc // 4)
                        off = 512 * IB - i0
                        nc.tensor.matmul(
                            ps_y[:, :],
                            lhsT=vj[p][jc][:, 66 * s : 66 * s + 65],
                            rhs=pts[jc][:, off : off + 512],
                            start=(jc == 0),
                            stop=(jc == njc - 1),
                        )
                    rs = rsp.tile([1, 512], F32, tag="rs", name=f"rs{u}_{IB}")
                    nc.vector.reciprocal(rs[:, :], ps_y[64:65, :])
                    rb = rsp.tile([64, 512], F32, tag="rb", name=f"rb{u}_{IB}")
                    nc.gpsimd.partition_broadcast(rb[:, :], rs[0:1, :])
                    nc.vector.scalar_tensor_tensor(
                        out=yT[p][sl, bass.ts(IB, 512)],
                        in0=ps_y[0:64, :],
                        scalar=1.0,
                        in1=rb[:, :],
                        op0=BYPASS,
                        op1=MULT,
                    )
                    if proj_cb is not None:
                        proj_cb(IB)

            def emit_proj(IB):
                for i128 in range(4 * IB, 4 * IB + 4):
                    o_t = outp.tile([128, 1024], F16, tag="out", name=f"out{i128}")
                    for eb in range(2):
                        ps_o = stps.tile([128, 512], F32, tag="st", name=f"proj{i128}_{eb}")
                        for p in range(2):
                            nc.tensor.matmul(
                                ps_o[:, :],
                                lhsT=yT[p][:, bass.ts(i128, 128)],
                                rhs=wproj_sb[:, p, bass.ts(eb, 512)],
                                start=(p == 0),
                                stop=(p == 1),
                            )
                        nc.scalar.activation(o_t[:, bass.ts(eb, 512)], ps_o[:, :], COPY)
                    nc.sync.dma_start(out_d.ap()[bass.ts(i128, 128), :], o_t[:, :])

            prev = None
            for u in range(4):
                pts = emit_st(u, ADD_ENGINE[u])
                if prev is not None:
                    emit_attv(*prev)
                prev = (u, pts)
            emit_attv(*prev, proj_cb=emit_proj)

    nc.compile()
    return nc


_NC_CACHE = None
RUN_KWARGS = {}
LAST_RESULTS = None
LAST_IN_MAPS = None


def _get_program():
    global _NC_CACHE
    if _NC_CACHE is None:
        _NC_CACHE = _build_program()
    return _NC_CACHE


def kernel(x, Wqkv, bqkv, Wproj, bproj, Er):
    x = np.asarray(x, dtype=np.float32)
    Wqkv = np.asarray(Wqkv, dtype=np.float32)
    bqkv = np.asarray(bqkv, dtype=np.float32)
    Wproj = np.asarray(Wproj, dtype=np.float32)
    bproj = np.asarray(bproj, dtype=np.float32)
    Er = np.asarray(Er, dtype=np.float32)

    nc = _get_program()

    start = BLOCK_SIZE - L
    erT = np.ascontiguousarray(Er[start:, :].T).astype(np.float16)

    in_maps = []
    for c in range(N_CORES):
        b = c // 4
        h0 = HPC * (c % 4)
        xT = np.ascontiguousarray(x[b].T).astype(np.float16)
        wq = np.empty((2, D, 128), np.float16)
        wk = np.empty((2, D, 128), np.float16)
        wv = np.empty((2, D, 128), np.float16)
        bq = np.empty((2, 128, 1), np.float32)
        bk = np.empty((2, 128, 1), np.float32)
        wproj = np.empty((2, 128, D), np.float16)
        for p in range(2):
            c0 = (h0 + 2 * p) * HS
            wq[p] = Wqkv[:, c0 : c0 + 128]
            wk[p] = Wqkv[:, D + c0 : D + c0 + 128]
            wv[p] = Wqkv[:, 2 * D + c0 : 2 * D + c0 + 128]
            bq[p, :, 0] = bqkv[c0 : c0 + 128]
            bk[p, :, 0] = bqkv[D + c0 : D + c0 + 128]
            wproj[p] = Wproj[c0 : c0 + 128, :].astype(np.float16)
        in_maps.append(
            {"xT": xT, "wq": wq, "wk": wk, "wv": wv, "bq": bq, "bk": bk,
             "erT": erT, "wproj": wproj}
        )

    global LAST_RESULTS, LAST_IN_MAPS
    LAST_IN_MAPS = in_maps
    res = run_bass_kernel_spmd(nc, in_maps, core_ids=list(range(N_CORES)), **RUN_KWARGS)
    LAST_RESULTS = res

    bv = bqkv[2 * D :]
    bias_vec = bv @ Wproj + bproj
    out = np.zeros((B, L, D), np.float32)
    for c in range(N_CORES):
        out[c // 4] += res.results[c]["out"].astype(np.float32)
    out += bias_vec[None, None, :]
    return out
